# revision 11
# baseline (speedup 1.0000x reference)
"""Expert-parallel grouped matmul (MoE BatchLinear) for 8 Trainium2 NeuronCores.

Problem: y[t] = x[t] @ W[g(t)] where tokens are grouped contiguously by expert
g (G=64 experts, counts given at runtime). Sharding: expert-parallel — core c
owns experts [8c, 8c+8) and the contiguous token rows routed to them. The
"all-to-all" is done host-side: kernel() receives full inputs, slices/pads
per-core token blocks, and scatters per-core outputs back.

Device kernel (SPMD, one program on 8 cores):
  for each local expert e (8 per core, big-expert-first order):
    per-m-tile xT tiles [128ki, 8ko, 128] streamed on the sync ring
    for each W chunk (ladder widths on first/last expert, else 2048):
      W chunk resident as [128ki, 8ko, w] (own contiguous DRAM tensor)
      for each 128-token m-tile:
        8 k-steps x nb matmuls (N<=512) accumulate into nb PSUM banks
        DVE copy PSUM -> SBUF fp16 staging, ACT-ring DMA staging -> y

All DRAM blocks are fully contiguous per partition: each W chunk is its own
[ki, ko, w] tensor, xT is per-expert [mt, ki, ko, 128], y is [mtile, 128,
D_OUT] written at ascending column offsets.

Numerics: operands stream as fp16 (1 PE cycle/row, fp32 PSUM accumulation);
y returns as fp16 and is upcast host-side (adds <~5e-4 absmax vs fp32 out).
"""

import numpy as np

G, N_TOK, D_IN, D_OUT, CAP = 64, 32768, 1024, 4096, 768
M_CORES = 8
EPC = G // M_CORES          # experts per core
P = 128                     # partitions / k-tile / m-tile
KO = D_IN // P              # 8 k-tiles
ORDER = "bigfirst"          # "bigfirst" | "natural"

_cache = {}


def _slot_order(mt):
    """Expert processing order. bigfirst interleaves big/small (768,256,...)
    so the first expert's long compute builds W-prefetch credit."""
    alive = [j for j in range(EPC) if mt[j] > 0]
    if ORDER == "natural":
        return alive
    big = sorted(alive, key=lambda s: -mt[s])
    bigs = big[: (len(alive) + 1) // 2]
    small = [s for s in alive if s not in bigs]
    # stable interleave: biggest, smallest-partner, next-biggest, ...
    out = []
    for a, b in zip(bigs, small + [None]):
        out.append(a)
        if b is not None:
            out.append(b)
    if len(bigs) > len(small) + 1:
        out += bigs[len(small) + 1:]
    return out


def _widths(ei, n):
    """W chunk widths per expert: ascending ladder on the first expert (small
    time-to-first-matmul), descending on the last (small drain tail)."""
    lead, tail = ei == 0, ei == n - 1
    if lead and tail:
        return [256, 256, 512, 1024, 1024, 512, 256, 256]
    if lead:
        return [256, 256, 512, 1024, 1024, 1024]
    if tail:
        return [1024, 1024, 1024, 512, 256, 256]
    return [1024, 1024, 1024, 1024]


def _build(mt):
    """Compile the SPMD program for per-expert-slot m-tile counts mt (len EPC)."""
    import concourse.mybir as mybir
    import concourse.tile as tile
    from concourse import bacc

    f32 = mybir.dt.float32
    f16 = mybir.dt.float16
    n_mtiles = sum(mt)
    order = _slot_order(mt)
    ne = len(order)

    nc = bacc.Bacc("TRN2", target_bir_lowering=False, debug=False)
    xt_d = {
        e: nc.dram_tensor(f"xT{e}", [mt[e], P, KO, P], f16, kind="ExternalInput")
        for e in order
    }
    w_d = {
        (e, ci): nc.dram_tensor(f"W{e}_{ci}", [P, KO, wd], f16, kind="ExternalInput")
        for ei, e in enumerate(order)
        for ci, wd in enumerate(_widths(ei, ne))
    }
    y_d = nc.dram_tensor("y", [n_mtiles, P, D_OUT], f16, kind="ExternalOutput")
    y = y_d.ap()

    with tile.TileContext(nc) as tc:
        with (
            tc.tile_pool(name="wq", bufs=10) as wq_pool,
            tc.tile_pool(name="xt", bufs=8) as xt_pool,
            tc.tile_pool(name="st", bufs=10) as st_pool,
            tc.tile_pool(name="ps", bufs=8, space="PSUM") as ps_pool,
        ):
            mi0 = 0  # global m-tile index
            wqs = {}
            for ei, e in enumerate(order):
                xts = []
                for m in range(mt[e]):
                    xm = xt_pool.tile([P, KO, P], f16, tag="xt", name="xt")
                    xts.append(xm)

                def _load_xt(m):
                    nc.scalar.dma_start(out=xts[m][:], in_=xt_d[e].ap()[m])

                if ei == 0:
                    # first matmul is gated only by xt m-tile 0 + W chunk 0:
                    # m0 loads on the scalar ring while m1 + chunk 0 go out
                    # in parallel on the sync ring, then the remaining m-tiles
                    _load_xt(0)
                    wd0 = _widths(ei, ne)[0]
                    wq0 = wq_pool.tile([P, KO, wd0], f16, tag="wq", name="wq")
                    nc.sync.dma_start(out=wq0[:], in_=w_d[(e, 0)].ap())
                    wqs[0] = wq0
                    if mt[e] > 1:
                        nc.sync.dma_start(out=xts[1][:], in_=xt_d[e].ap()[1])
                    for m in range(2, mt[e]):
                        _load_xt(m)
                else:
                    for m in range(mt[e]):
                        _load_xt(m)
                col = 0
                for ci, wd in enumerate(_widths(ei, ne)):
                    if ei == 0 and ci == 0:
                        wq = wqs.pop(0)
                    else:
                        wq = wq_pool.tile([P, KO, wd], f16, tag="wq", name="wq")
                        nc.sync.dma_start(out=wq[:], in_=w_d[(e, ci)].ap())
                    nb = (wd + 511) // 512
                    for m in range(mt[e]):
                        pss = []
                        for nn in range(nb):
                            w_nn = min(512, wd - nn * 512)
                            pss.append(
                                ps_pool.tile([P, w_nn], f32, tag="ps", name="ps")
                            )
                        for k in range(KO):
                            lhsT = xts[m][:, k, :]
                            for nn in range(nb):
                                w_nn = min(512, wd - nn * 512)
                                nc.tensor.matmul(
                                    pss[nn][:],
                                    lhsT,
                                    wq[:, k, nn * 512 : nn * 512 + w_nn],
                                    start=(k == 0),
                                    stop=(k == KO - 1),
                                )
                        st = st_pool.tile([P, wd], f16, tag="st", name="st")
                        for nn in range(nb):
                            w_nn = min(512, wd - nn * 512)
                            nc.vector.tensor_copy(
                                st[:, nn * 512 : nn * 512 + w_nn], pss[nn][:]
                            )
                        nc.scalar.dma_start(
                            out=y[mi0 + m, :, col : col + wd], in_=st[:]
                        )
                    col += wd
                mi0 += mt[e]
    nc.compile()
    return nc


def _prepare(x, weight, counts):
    """Host-side all-to-all: per-core padded token blocks + weight chunks."""
    starts = np.zeros(G + 1, np.int64)
    np.cumsum(counts, out=starts[1:])
    cnt = counts.reshape(M_CORES, EPC)
    mt = tuple(int(v) for v in np.ceil(cnt / P).astype(np.int64).max(axis=0))

    order = _slot_order(mt)
    ne = len(order)
    in_maps, metas = [], []
    for c in range(M_CORES):
        im = {}
        meta = []
        mi0 = 0
        mi0_by_slot = {}
        for j in order:
            mi0_by_slot[j] = mi0
            mi0 += mt[j]
        for ji, j in enumerate(order):
            g = c * EPC + j
            s, n = int(starts[g]), int(counts[g])
            n = min(n, N_TOK - s) if s < N_TOK else 0
            te = P * mt[j]
            xe = np.zeros((te, D_IN), np.float16)
            if n > 0:
                xe[:n] = x[s : s + n]
            # [te, D_IN] -> [D_IN, te] -> [KO, P, mt, 128] -> [mt, P, KO, 128]
            im[f"xT{j}"] = np.ascontiguousarray(
                xe.T.reshape(KO, P, mt[j], P).transpose(2, 1, 0, 3)
            )
            # weight [D_IN, D_OUT] -> chunks [P, KO, w]
            wg = weight[g].reshape(KO, P, D_OUT).transpose(1, 0, 2).astype(np.float16)
            col = 0
            for ci, wd in enumerate(_widths(ji, ne)):
                im[f"W{j}_{ci}"] = np.ascontiguousarray(wg[:, :, col : col + wd])
                col += wd
            meta.append((mi0_by_slot[j], s, n))
        in_maps.append(im)
        metas.append(meta)
    return mt, in_maps, metas


def _ensure_axon_hooks_shim():
    """bass_utils imports antenv.axon_hooks when tracing is requested (e.g.
    via a BASS_TRACE env var); some images lack that module. Install a no-op
    shim so the run degrades to untraced instead of crashing."""
    try:
        from antenv.axon_hooks import get_axon_ntff_profile_hook  # noqa: F401
        return
    except ImportError:
        pass
    import sys
    import types

    try:
        import antenv
    except ImportError:
        return
    mod = types.ModuleType("antenv.axon_hooks")
    mod._hook = None
    mod.get_axon_ntff_profile_hook = lambda: getattr(mod, "_hook", None)

    def _set(h):
        mod._hook = h

    mod.set_axon_ntff_profile_hook = _set
    sys.modules["antenv.axon_hooks"] = mod
    antenv.axon_hooks = mod


def _run(x, weight, counts, trace=False, trace_cores=None):
    from concourse.bass_utils import run_bass_kernel_spmd

    _ensure_axon_hooks_shim()

    x = np.ascontiguousarray(np.asarray(x, dtype=np.float32))
    weight = np.ascontiguousarray(np.asarray(weight, dtype=np.float32))
    counts = np.asarray(counts).astype(np.int64)
    assert counts.shape == (G,)

    mt, in_maps, metas = _prepare(x, weight, counts)
    if sum(mt) == 0:
        return np.zeros((N_TOK, D_OUT), np.float32), None
    if mt not in _cache:
        _cache[mt] = _build(mt)
    nc = _cache[mt]

    res = run_bass_kernel_spmd(
        nc,
        in_maps,
        core_ids=list(range(M_CORES)),
        trace=trace,
        trace_cores=trace_cores,
    )
    out = np.zeros((N_TOK, D_OUT), np.float32)
    for c in range(M_CORES):
        yc = res.results[c]["y"]  # [n_mtiles, P, D_OUT] fp16
        n_mtiles = yc.shape[0]
        yc = yc.reshape(n_mtiles * P, D_OUT).astype(np.float32)
        for mi0, s, n in metas[c]:
            if n > 0:
                out[s : s + n] = yc[mi0 * P : mi0 * P + n]
    return out, res


def kernel(x, weight, num_inputs_per_group):
    out, _ = _run(x, weight, num_inputs_per_group)
    return out
